# revision 9
# baseline (speedup 1.0000x reference)
"""BinaryConv2D Trainium2 kernel.

Reference op: out = conv2d(sign(clip(x,-1,1)), sign(clip(w,-1,1))),
NHWC x HWIO -> NHWC, SAME padding, stride 1, fp32.

Since sign() of a nonzero float is exactly +-1 (exactly representable in
bf16), and every partial sum is an integer bounded by 3*3*256 = 2304
(< 2^24), the conv can be computed EXACTLY with bf16 matmuls
accumulating into fp32 PSUM.

Sharding: data-parallel over batch. 32 images / 8 cores = 4 images per
core; full weights replicated. No collectives.

Per-core pipeline:
  1. Binarize weights to bf16 tiles [cin=128 part, cout=256] (HWIO layout
     already has cin contiguous over rows -> no transpose needed).
  2. Per image: load raw fp32 pixels, sign->bf16 on the ACT engine, DMA
     back out to a zero-padded DRAM staging buffer laid out as a 58x58
     padded pixel grid (SAME padding becomes plain zero borders).
  3. One big DMA-transpose per (image, cin-chunk): staged [3392 pix, 128
     cin] bf16 -> SBUF act_T [128 cin, 3392 pix] (channel-major).
  4. Conv as implicit GEMM: psum[cout=128, 448 pix] accumulates 9 taps x
     2 cin chunks = 18 matmuls; rhs is a strided window into act_T
     ([[58,8],[1,56]] access pattern = 8 output rows x 56 cols).
  5. PSUM -> SBUF (DVE copy) -> DRAM out [2, 128, 12544] (cout-major);
     host transposes per-core results back to NHWC while unsharding.
"""

import numpy as np

import concourse.bass as bass
import concourse.mybir as mybir
from concourse import bacc
from concourse.tile import TileContext
from concourse.bass_utils import run_bass_kernel_spmd

F32 = mybir.dt.float32
BF16 = mybir.dt.bfloat16
FP8 = mybir.dt.float8e4

N_CORES = 8
N_IMG = 4            # images per core
H = W = 56
CIN = COUT = 256
NPIX = H * W                      # 3136 pixels per image
PW = W + 2                        # 58: padded row width
PH = H + 2                        # 58: padded rows
PIXPAD = PW * PH                  # 3364 padded pixels
PIXPAD_AL = 3392                  # aligned up to 16 for DMA transpose
CH = 3456                         # act chunk stride (room for AP construction)
ROWBLK = 8                        # output rows per psum tile
NBLK = H // ROWBLK                # 7
NTP = ROWBLK * W                  # 448 = psum free size
RAW_F = 896                       # fp32 elems per partition per raw tile
NRAW = (NPIX * CIN) // (128 * RAW_F)  # 7 raw tiles per image


def build(nc: bass.Bass, mode: str = "bf16"):
    x_d = nc.dram_tensor("x", [N_IMG * NPIX, CIN], F32, kind="ExternalInput")
    w_d = nc.dram_tensor("w", [9 * CIN, COUT], F32, kind="ExternalInput")
    y_d = nc.dram_tensor("y", [2, 128, N_IMG * NPIX], F32, kind="ExternalOutput")

    xf = x_d[:].rearrange("r c -> (r c)")          # flat fp32 input
    xt = xf.rearrange("(t p f) -> t p f", p=128, f=RAW_F)  # [28, 128, 896]
    NT = 464 if mode == "fp8" else NTP             # psum free size

    with TileContext(nc) as tc:
        with (
            tc.tile_pool(name="wpool", bufs=1) as wpool,
            tc.tile_pool(name="wstage", bufs=2) as wstage,
            tc.tile_pool(name="zpool", bufs=1) as zpool,
            tc.tile_pool(name="stage", bufs=2, space="DRAM") as dpool,
            tc.tile_pool(name="raw", bufs=3) as rawpool,
            tc.tile_pool(name="xb", bufs=3) as xbpool,
            tc.tile_pool(name="act", bufs=2) as actpool,
            tc.tile_pool(name="psum", bufs=4, space="PSUM") as psumpool,
            tc.tile_pool(name="out", bufs=4) as outpool,
        ):
            # ---- weights: binarize, [cin 128, cout 256] per (tap, chunk)
            if mode == "fp8":
                # DoubleRow block pairing: partition p holds cin (i*128+p)
                wb8 = wpool.tile([128, 9, 2, COUT], FP8)
                for t in range(9):
                    wst = wstage.tile([128, 2, COUT], F32)
                    nc.sync.dma_start(
                        out=wst[:],
                        in_=w_d[t * CIN : (t + 1) * CIN, :].rearrange(
                            "(i p) c -> p i c", i=2
                        ),
                    )
                    nc.scalar.sign(wb8[:, t], wst[:])
            else:
                wb = wpool.tile([128, 18 * COUT], BF16)
                for t in range(9):
                    for ki in range(2):
                        idx = t * 2 + ki
                        wst = wstage.tile([128, COUT], F32)
                        nc.sync.dma_start(
                            out=wst[:],
                            in_=w_d[t * CIN + ki * 128 : t * CIN + (ki + 1) * 128, :],
                        )
                        nc.scalar.sign(wb[:, idx * COUT : (idx + 1) * COUT], wst[:])

            # ---- zeros for padding borders
            zt = zpool.tile([57, 512], BF16)
            nc.gpsimd.memset(zt[:], 0.0)

            for n in range(N_IMG):
                stage = dpool.tile([PIXPAD_AL, CIN], BF16)
                sflat = stage[:].rearrange("r c -> (r c)")
                # zero borders: top pad row
                nc.sync.dma_start(
                    out=sflat[0 : PW * CIN].rearrange("(a b) -> a b", b=512),
                    in_=zt[0:29, :],
                )
                # bottom pad row
                nc.sync.dma_start(
                    out=sflat[57 * PW * CIN : 58 * PW * CIN].rearrange(
                        "(a b) -> a b", b=512
                    ),
                    in_=zt[0:29, :],
                )
                # side pads: right-pad of row r + left-pad of row r+1, r=0..56
                nc.sync.dma_start(
                    out=sflat[57 * CIN : 57 * CIN + 57 * PW * CIN]
                    .rearrange("(r x) -> r x", x=PW * CIN)[:, 0:512],
                    in_=zt[:, :],
                )
                # alignment tail rows (read by the transpose, never by matmul)
                nc.sync.dma_start(
                    out=sflat[PIXPAD * CIN : PIXPAD_AL * CIN].rearrange(
                        "(a b) -> a b", b=512
                    ),
                    in_=zt[0:14, :],
                )

                # load + binarize + store to padded staging
                for j in range(NRAW):
                    raw = rawpool.tile([128, RAW_F], F32)
                    nc.sync.dma_start(out=raw[:], in_=xt[n * NRAW + j])
                    xb = xbpool.tile([128, RAW_F], BF16)
                    nc.scalar.sign(xb[:], raw[:])
                    # 8 image rows -> padded rows 8j+1..8j+8, col offset 1
                    off = ((ROWBLK * j + 1) * PW + 1) * CIN
                    dst = sflat[off : off + ROWBLK * PW * CIN].rearrange(
                        "(r x) -> r x", x=PW * CIN
                    )[:, 0 : W * CIN]
                    nc.sync.dma_start(out=dst, in_=xb[:])

                # transpose to channel-major act_T [128 cin, pix]
                if mode == "fp8":
                    act8 = actpool.tile([128, 2, CH], FP8)
                    for ki in range(2):
                        actb = xbpool.tile(
                            [128, PIXPAD_AL], BF16, tag="actb", bufs=3
                        )
                        nc.scalar.dma_start(
                            out=actb[:],
                            in_=stage[:, ki * 128 : (ki + 1) * 128],
                            transpose=True,
                        )
                        nc.vector.tensor_copy(act8[:, ki, 0:PIXPAD_AL], actb[:])
                else:
                    act = actpool.tile([128, 2 * CH], BF16)
                    for ki in range(2):
                        nc.scalar.dma_start(
                            out=act[:, ki * CH : ki * CH + PIXPAD_AL],
                            in_=stage[:, ki * 128 : (ki + 1) * 128],
                            transpose=True,
                        )

                # conv matmuls
                for m in range(2):          # cout chunk
                    for j in range(NBLK):   # 8-row output block
                        psum = psumpool.tile([128, NT], F32)
                        if mode == "fp8":
                            for t in range(9):
                                dy, dx = t // 3 - 1, t % 3 - 1
                                base = (ROWBLK * j + 1 + dy) * PW + 1 + dx
                                rhs = act8[:, :, base : base + NT]
                                lhsT = wb8[:, t, :, m * 128 : (m + 1) * 128]
                                nc.tensor.matmul(
                                    psum[:],
                                    lhsT,
                                    rhs,
                                    start=(t == 0),
                                    stop=(t == 8),
                                    perf_mode=mybir.MatmulPerfMode.DoubleRow,
                                )
                        else:
                            first = True
                            for ki in range(2):
                                for t in range(9):
                                    dy, dx = t // 3 - 1, t % 3 - 1
                                    base = (
                                        ki * CH
                                        + (ROWBLK * j + 1 + dy) * PW
                                        + 1
                                        + dx
                                    )
                                    rhs = (
                                        act[:, base : base + ROWBLK * PW]
                                        .rearrange("p (r c) -> p r c", c=PW)[:, :, 0:W]
                                    )
                                    idx = t * 2 + ki
                                    lhsT = wb[
                                        :, idx * COUT + m * 128 : idx * COUT + (m + 1) * 128
                                    ]
                                    nc.tensor.matmul(
                                        psum[:],
                                        lhsT,
                                        rhs,
                                        start=first,
                                        stop=(ki == 1 and t == 8),
                                    )
                                    first = False
                        ot = outpool.tile([128, NTP], F32)
                        if mode == "fp8":
                            nc.vector.tensor_copy(
                                ot[:].rearrange("p (r c) -> p r c", c=W),
                                psum[:].rearrange("p (r c) -> p r c", c=PW)[:, :, 0:W],
                            )
                        else:
                            nc.vector.tensor_copy(ot[:], psum[:])
                        nc.scalar.dma_start(
                            out=y_d[m][:, n * NPIX + j * NTP : n * NPIX + (j + 1) * NTP],
                            in_=ot[:],
                        )
    return nc


def _run(x: np.ndarray, w: np.ndarray, trace: bool = False, mode: str = "bf16"):
    """x: (32,56,56,256) f32, w: (3,3,256,256) f32 -> (out, BassKernelResults)."""
    nc = bacc.Bacc(None, target_bir_lowering=False, debug=False)
    build(nc, mode=mode)
    nc.finalize()  # Bacc.compile: legalizes multi-wait insts into event sems
    wf = np.ascontiguousarray(w.reshape(9 * CIN, COUT))
    in_maps = []
    for c in range(N_CORES):
        xs = np.ascontiguousarray(
            x[c * N_IMG : (c + 1) * N_IMG].reshape(N_IMG * NPIX, CIN)
        )
        in_maps.append({"x": xs, "w": wf})
    res = run_bass_kernel_spmd(nc, in_maps, core_ids=list(range(N_CORES)), trace=trace)
    outs = []
    for c in range(N_CORES):
        y = res.results[c]["y"]  # [2, 128, 12544]
        o = (
            y.reshape(2, 128, N_IMG, H, W)
            .transpose(2, 3, 4, 0, 1)
            .reshape(N_IMG, H, W, COUT)
        )
        outs.append(o)
    return np.concatenate(outs, axis=0).astype(np.float32), res


def kernel(**inputs) -> np.ndarray:
    x = np.asarray(inputs["inputs"], dtype=np.float32)
    w = np.asarray(inputs["kernel"], dtype=np.float32)
    out, _ = _run(x, w, trace=False)
    return out


# revision 12
# speedup vs baseline: 1.3425x; 1.3425x over previous
"""BinaryConv2D Trainium2 kernel.

Reference op: out = conv2d(sign(clip(x,-1,1)), sign(clip(w,-1,1))),
NHWC x HWIO -> NHWC, SAME padding, stride 1, fp32.

Since sign() of a nonzero float is exactly +-1 (exactly representable in
bf16), and every partial sum is an integer bounded by 3*3*256 = 2304
(< 2^24), the conv can be computed EXACTLY with bf16 matmuls
accumulating into fp32 PSUM.

Sharding: data-parallel over batch. 32 images / 8 cores = 4 images per
core; full weights replicated. No collectives.

Per-core pipeline:
  1. Binarize weights to bf16 tiles [cin=128 part, cout=256] (HWIO layout
     already has cin contiguous over rows -> no transpose needed).
  2. Per image: load raw fp32 pixels, sign->bf16 on the ACT engine, DMA
     back out to a zero-padded DRAM staging buffer laid out as a 58x58
     padded pixel grid (SAME padding becomes plain zero borders).
  3. One big DMA-transpose per (image, cin-chunk): staged [3392 pix, 128
     cin] bf16 -> SBUF act_T [128 cin, 3392 pix] (channel-major).
  4. Conv as implicit GEMM: psum[cout=128, 448 pix] accumulates 9 taps x
     2 cin chunks = 18 matmuls; rhs is a strided window into act_T
     ([[58,8],[1,56]] access pattern = 8 output rows x 56 cols).
  5. PSUM -> SBUF (DVE copy) -> DRAM out [2, 128, 12544] (cout-major);
     host transposes per-core results back to NHWC while unsharding.
"""

import numpy as np

import concourse.bass as bass
import concourse.mybir as mybir
from concourse import bacc
from concourse.tile import TileContext
from concourse.bass_utils import run_bass_kernel_spmd

F32 = mybir.dt.float32
BF16 = mybir.dt.bfloat16
FP8 = mybir.dt.float8e4

N_CORES = 8
N_IMG = 4            # images per core
H = W = 56
CIN = COUT = 256
NPIX = H * W                      # 3136 pixels per image
PW = W + 2                        # 58: padded row width
PH = H + 2                        # 58: padded rows
PIXPAD = PW * PH                  # 3364 padded pixels
PIXPAD_AL = 3392                  # aligned up to 16 for DMA transpose
CH = 3456                         # act chunk stride (room for AP construction)
ROWBLK = 8                        # output rows per psum tile
NBLK = H // ROWBLK                # 7
NTP = ROWBLK * W                  # 448 = psum free size
RAW_F = 896                       # fp32 elems per partition per raw tile
NRAW = (NPIX * CIN) // (128 * RAW_F)  # 7 raw tiles per image


def build(nc: bass.Bass, mode: str = "bf16"):
    x_d = nc.dram_tensor("x", [N_IMG * NPIX, CIN], F32, kind="ExternalInput")
    w_d = nc.dram_tensor("w", [9 * CIN, COUT], F32, kind="ExternalInput")
    y_d = nc.dram_tensor("y", [2, 128, N_IMG * NPIX], F32, kind="ExternalOutput")

    xf = x_d[:].rearrange("r c -> (r c)")          # flat fp32 input
    xt = xf.rearrange("(t p f) -> t p f", p=128, f=RAW_F)  # [28, 128, 896]
    NT = 464 if mode == "fp8" else NTP             # psum free size

    with TileContext(nc) as tc:
        with (
            tc.tile_pool(name="wpool", bufs=1) as wpool,
            tc.tile_pool(name="wstage", bufs=2) as wstage,
            tc.tile_pool(name="zpool", bufs=1) as zpool,
            tc.tile_pool(name="stage", bufs=2, space="DRAM") as dpool,
            tc.tile_pool(name="raw", bufs=3) as rawpool,
            tc.tile_pool(name="xb", bufs=3) as xbpool,
            tc.tile_pool(name="act", bufs=2) as actpool,
            tc.tile_pool(name="psum", bufs=4, space="PSUM") as psumpool,
            tc.tile_pool(name="out", bufs=4) as outpool,
        ):
            # ---- weights: binarize, [cin 128, cout 256] per (tap, chunk)
            if mode == "fp8":
                # DoubleRow block pairing: partition p holds cin (i*128+p)
                wb8 = wpool.tile([128, 9, 2, COUT], FP8)
                for t in range(9):
                    wst = wstage.tile([128, 2, COUT], F32)
                    nc.gpsimd.dma_start(
                        out=wst[:],
                        in_=w_d[t * CIN : (t + 1) * CIN, :].rearrange(
                            "(i p) c -> p i c", i=2
                        ),
                    )
                    nc.scalar.sign(wb8[:, t], wst[:])
            else:
                wb = wpool.tile([128, 18 * COUT], BF16)
                for t in range(9):
                    for ki in range(2):
                        idx = t * 2 + ki
                        wst = wstage.tile([128, COUT], F32)
                        nc.gpsimd.dma_start(
                            out=wst[:],
                            in_=w_d[t * CIN + ki * 128 : t * CIN + (ki + 1) * 128, :],
                        )
                        nc.scalar.sign(wb[:, idx * COUT : (idx + 1) * COUT], wst[:])

            # ---- zeros for padding borders
            zt = zpool.tile([57, 512], BF16)
            nc.gpsimd.memset(zt[:], 0.0)

            def prep(n):
                """Stage image n: load, binarize, pad, transpose to act_T."""
                stage = dpool.tile([PIXPAD_AL, CIN], BF16, tag="stage")
                sflat = stage[:].rearrange("r c -> (r c)")
                # zero borders: top pad row
                nc.gpsimd.dma_start(
                    out=sflat[0 : PW * CIN].rearrange("(a b) -> a b", b=512),
                    in_=zt[0:29, :],
                )
                # bottom pad row
                nc.gpsimd.dma_start(
                    out=sflat[57 * PW * CIN : 58 * PW * CIN].rearrange(
                        "(a b) -> a b", b=512
                    ),
                    in_=zt[0:29, :],
                )
                # side pads: right-pad of row r + left-pad of row r+1, r=0..56
                nc.gpsimd.dma_start(
                    out=sflat[57 * CIN : 57 * CIN + 57 * PW * CIN]
                    .rearrange("(r x) -> r x", x=PW * CIN)[:, 0:512],
                    in_=zt[:, :],
                )
                # alignment tail rows (read by the transpose, never by matmul)
                nc.gpsimd.dma_start(
                    out=sflat[PIXPAD * CIN : PIXPAD_AL * CIN].rearrange(
                        "(a b) -> a b", b=512
                    ),
                    in_=zt[0:14, :],
                )

                # load + binarize + store to padded staging
                for j in range(NRAW):
                    raw = rawpool.tile([128, RAW_F], F32, tag="raw")
                    nc.sync.dma_start(out=raw[:], in_=xt[n * NRAW + j])
                    xb = xbpool.tile([128, RAW_F], BF16, tag="xb")
                    nc.scalar.sign(xb[:], raw[:])
                    # 8 image rows -> padded rows 8j+1..8j+8, col offset 1
                    off = ((ROWBLK * j + 1) * PW + 1) * CIN
                    dst = sflat[off : off + ROWBLK * PW * CIN].rearrange(
                        "(r x) -> r x", x=PW * CIN
                    )[:, 0 : W * CIN]
                    nc.sync.dma_start(out=dst, in_=xb[:])

                # transpose to channel-major act_T [128 cin, pix]
                if mode == "fp8":
                    act8 = actpool.tile([128, 2, CH], FP8, tag="act8")
                    for ki in range(2):
                        actb = xbpool.tile(
                            [128, PIXPAD_AL], BF16, tag="actb", bufs=3
                        )
                        nc.scalar.dma_start(
                            out=actb[:],
                            in_=stage[:, ki * 128 : (ki + 1) * 128],
                            transpose=True,
                        )
                        nc.vector.tensor_copy(act8[:, ki, 0:PIXPAD_AL], actb[:])
                    return act8
                act = actpool.tile([128, 2 * CH], BF16, tag="act")
                for ki in range(2):
                    nc.scalar.dma_start(
                        out=act[:, ki * CH : ki * CH + PIXPAD_AL],
                        in_=stage[:, ki * 128 : (ki + 1) * 128],
                        transpose=True,
                    )
                return act

            acts = {0: prep(0)}
            for n in range(N_IMG):
                if n + 1 < N_IMG:
                    acts[n + 1] = prep(n + 1)
                if mode == "fp8":
                    act8 = acts[n]
                else:
                    act = acts[n]
                # conv matmuls
                for m in range(2):          # cout chunk
                    for j in range(NBLK):   # 8-row output block
                        psum = psumpool.tile([128, NT], F32)
                        if mode == "fp8":
                            for t in range(9):
                                dy, dx = t // 3 - 1, t % 3 - 1
                                base = (ROWBLK * j + 1 + dy) * PW + 1 + dx
                                rhs = act8[:, :, base : base + NT]
                                lhsT = wb8[:, t, :, m * 128 : (m + 1) * 128]
                                nc.tensor.matmul(
                                    psum[:],
                                    lhsT,
                                    rhs,
                                    start=(t == 0),
                                    stop=(t == 8),
                                    perf_mode=mybir.MatmulPerfMode.DoubleRow,
                                )
                        else:
                            first = True
                            for ki in range(2):
                                for t in range(9):
                                    dy, dx = t // 3 - 1, t % 3 - 1
                                    base = (
                                        ki * CH
                                        + (ROWBLK * j + 1 + dy) * PW
                                        + 1
                                        + dx
                                    )
                                    rhs = (
                                        act[:, base : base + ROWBLK * PW]
                                        .rearrange("p (r c) -> p r c", c=PW)[:, :, 0:W]
                                    )
                                    idx = t * 2 + ki
                                    lhsT = wb[
                                        :, idx * COUT + m * 128 : idx * COUT + (m + 1) * 128
                                    ]
                                    nc.tensor.matmul(
                                        psum[:],
                                        lhsT,
                                        rhs,
                                        start=first,
                                        stop=(ki == 1 and t == 8),
                                    )
                                    first = False
                        ot = outpool.tile([128, NTP], F32)
                        if mode == "fp8":
                            nc.vector.tensor_copy(
                                ot[:].rearrange("p (r c) -> p r c", c=W),
                                psum[:].rearrange("p (r c) -> p r c", c=PW)[:, :, 0:W],
                            )
                        else:
                            nc.vector.tensor_copy(ot[:], psum[:])
                        nc.gpsimd.dma_start(
                            out=y_d[m][:, n * NPIX + j * NTP : n * NPIX + (j + 1) * NTP],
                            in_=ot[:],
                        )
    return nc


def _run(x: np.ndarray, w: np.ndarray, trace: bool = False, mode: str = "bf16"):
    """x: (32,56,56,256) f32, w: (3,3,256,256) f32 -> (out, BassKernelResults)."""
    nc = bacc.Bacc(None, target_bir_lowering=False, debug=False)
    build(nc, mode=mode)
    nc.finalize()  # Bacc.compile: legalizes multi-wait insts into event sems
    wf = np.ascontiguousarray(w.reshape(9 * CIN, COUT))
    in_maps = []
    for c in range(N_CORES):
        xs = np.ascontiguousarray(
            x[c * N_IMG : (c + 1) * N_IMG].reshape(N_IMG * NPIX, CIN)
        )
        in_maps.append({"x": xs, "w": wf})
    res = run_bass_kernel_spmd(nc, in_maps, core_ids=list(range(N_CORES)), trace=trace)
    outs = []
    for c in range(N_CORES):
        y = res.results[c]["y"]  # [2, 128, 12544]
        o = (
            y.reshape(2, 128, N_IMG, H, W)
            .transpose(2, 3, 4, 0, 1)
            .reshape(N_IMG, H, W, COUT)
        )
        outs.append(o)
    return np.concatenate(outs, axis=0).astype(np.float32), res


def kernel(**inputs) -> np.ndarray:
    x = np.asarray(inputs["inputs"], dtype=np.float32)
    w = np.asarray(inputs["kernel"], dtype=np.float32)
    out, _ = _run(x, w, trace=False)
    return out
